# revision 14
# baseline (speedup 1.0000x reference)
"""Trainium2 Bass kernel for BlockChunkedActivityRoutedNet.

Reference computation (B=4096, IN_F=4096, 8 chunks of 512, top-2 by mean|x|,
chunk-expert Linears 512->512, concat -> final Linear 1024->4096):

    xr = x.reshape(B, 8, 512)
    activities = mean(|xr|, axis=(0, 2))            # over the WHOLE batch
    i0, i1 = top2(activities)                        # descending
    h = concat(xr[:, i0] @ Wc[i0] + bc[i0], xr[:, i1] @ Wc[i1] + bc[i1])
    out = h @ W_final + b_final

Distribution: data-parallel over the batch across 8 NeuronCores (512 rows
each). Per-chunk |x| partial sums are AllGathered (tiny 32B collective) so
every core computes the identical top-2 routing.

Schedule (v5). Measured on HW: the runtime's pre-collective barrier +
ncfw gap + collective is a ~60-90us fixed pipeline that starts ~21us into
every execution regardless of kernel code, so every engine has a large
guaranteed-idle window before routing can possibly be known. v5 exploits it:

  - Head: x loaded straight into MATMUL k-tile layout (xc[c][p, kt*512+b] =
    x.T[c*512+kt*128+p, b]); the per-chunk |.| activity reduce is layout-
    independent because the ones-matmul partition reduce sums everything
    anyway. Reduces split DVE/ScalarE; partition-reduce matmul; the whole
    trigger chain runs under tc.high_priority() so the list scheduler
    cannot park 128 L1 matmuls ahead of it. AllGather trigger ~25us.
  - Free window: L1 is computed for ALL 8 chunks (128 matmuls, PE would
    otherwise idle), biases added at eviction with compile-time chunk
    indices, and hT_all [4096, 512] bf16 is written to internal DRAM.
    W_final (8 x 1MB), b_final broadcast, bT also prefetch here.
  - Post-AllGather: partition-sum of the 8x8 partials (one matmul), top2
    via max/max_index, K=1 broadcast matmul, iota offsets; then the ONLY
    routing-dependent data movement: 8 indirect row-gathers of hT_all
    (hsel[s][d], 128KB each). No x/W gather, no second L1 pass.
  - L2 starts on the FIRST gathered tile: six PSUM groups run kf-outer at
    gather pace (6 MMs per landed hsel tile ~= the gather cadence), so the
    gather latency hides under the dense stream; remaining 26 groups run
    kf-inner. Evictions add b_final on DVE, cast to bf16 into a [128,4096]
    staging row per bt, stored as 4 x 1MB DMAs. Host casts fp32.
"""

import numpy as np
import ml_dtypes

import concourse.bass as bass
import concourse.bacc as bacc
import concourse.mybir as mybir
from concourse.tile import TileContext
from concourse.bass_utils import run_bass_kernel_spmd

dt = mybir.dt
P = 128

NUM_CHUNKS = 8
TOP_K = 2
IN_F = 4096
HID_F = 4096
OUT_F = 4096
B = 4096
CIN = IN_F // NUM_CHUNKS      # 512
COUT = HID_F // NUM_CHUNKS    # 512
N_CORES = 8
BS = B // N_CORES             # 512 rows per core

BT = BS // P                  # 4 batch tiles per core
KT = CIN // P                 # 4 k-tiles per selected chunk
DT_ = COUT // P               # 4 d-tiles per selected chunk
KF = TOP_K * DT_              # 8 k-tiles for the final matmul
OT = OUT_F // 512             # 8 output column tiles of 512

_cache = {}


def _build():
    nc = bacc.Bacc(num_devices=N_CORES, name="chunk_routed_net",
                   num_swdge_queues=1)

    xT = nc.dram_tensor("xT_shard", [IN_F, BS], dt.bfloat16,
                        kind="ExternalInput")
    Wc_n = nc.dram_tensor("Wc_rows", [IN_F, COUT], dt.bfloat16,
                          kind="ExternalInput")
    bT_d = nc.dram_tensor("bT_host", [P, DT_ * NUM_CHUNKS], dt.float32,
                          kind="ExternalInput")
    Wf = nc.dram_tensor("W_final", [COUT * TOP_K, OUT_F], dt.bfloat16,
                        kind="ExternalInput")
    bfb = nc.dram_tensor("b_final_bc", [P, OUT_F], dt.float32,
                         kind="ExternalInput")
    out = nc.dram_tensor("out_shard", [BS, OUT_F], dt.bfloat16,
                         kind="ExternalOutput")

    with TileContext(nc) as tc:
        with tc.tile_pool(name="consts", bufs=1) as consts, \
             tc.tile_pool(name="route", bufs=1) as route, \
             tc.tile_pool(name="bfinp", bufs=1) as bfinp, \
             tc.tile_pool(name="wfs", bufs=8) as wfs, \
             tc.tile_pool(name="dram", bufs=1, space="DRAM") as dram:

            # ---------------- constants ----------------
            ones_col = consts.tile([P, 1], dt.float32)     # partition reduce
            nc.vector.memset(ones_col[:], 1.0)
            ones8 = consts.tile([N_CORES, P], dt.float32)  # sum+bcast matmul
            nc.vector.memset(ones8[:], 1.0)
            # C1[p] = p   (row offset within a d-tile row block)
            C_W = consts.tile([P, 1], dt.int32)
            nc.gpsimd.iota(C_W[:], pattern=[[P, 1]], base=0, channel_multiplier=1)
            C_Wf = consts.tile([P, 1], dt.float32)
            nc.vector.tensor_copy(C_Wf[:], C_W[:])

            hT_all = dram.tile([NUM_CHUNKS * P, KT * BS], dt.bfloat16)
            cc_in = dram.tile([1, NUM_CHUNKS], dt.float32)
            cc_out = dram.tile([N_CORES, NUM_CHUNKS], dt.float32)

            with tc.tile_pool(name="xcp", bufs=1) as xcp, \
                 tc.tile_pool(name="wcp", bufs=1) as wcp, \
                 tc.tile_pool(name="hts", bufs=1) as hts, \
                 tc.tile_pool(name="ps_e1", bufs=1, space="PSUM") as ps_e1, \
                 tc.tile_pool(name="ps_h", bufs=2, space="PSUM") as ps_h:

                # ---- phase 1: loads + activities + trigger + L1-all ----
                xcs, wcs = [], []
                for c in range(NUM_CHUNKS):
                    xc = xcp.tile([P, KT * BS], dt.bfloat16, tag=f"xc{c}",
                                  name=f"xc{c}")
                    nc.sync.dma_start(
                        xc[:].rearrange("p (g b) -> p g b", g=KT),
                        xT[c * CIN:(c + 1) * CIN, :].rearrange(
                            "(g p) b -> p g b", p=P))
                    xcs.append(xc)
                for c in range(NUM_CHUNKS):
                    wc = wcp.tile([P, KT * COUT], dt.bfloat16, tag=f"wc{c}",
                                  name=f"wc{c}")
                    nc.sync.dma_start(
                        wc[:].rearrange("p (g d) -> p g d", g=KT),
                        Wc_n[c * CIN:(c + 1) * CIN, :].rearrange(
                            "(g p) d -> p g d", p=P))
                    wcs.append(wc)

                actcol = route.tile([P, NUM_CHUNKS], dt.float32)
                scr = route.tile([P, KT * BS], dt.bfloat16)  # ACT throwaway
                with tc.high_priority():
                    for c in range(NUM_CHUNKS):
                        if c % 2 == 0:
                            nc.vector.tensor_reduce(
                                actcol[:, c:c + 1], xcs[c][:],
                                axis=mybir.AxisListType.X,
                                op=mybir.AluOpType.add,
                                apply_absolute_value=True)
                        else:
                            nc.scalar.activation(
                                scr[:], xcs[c][:],
                                mybir.ActivationFunctionType.Abs,
                                accum_out=actcol[:, c:c + 1])
                    act_ps = ps_e1.tile([1, NUM_CHUNKS], dt.float32, tag="psa")
                    nc.tensor.matmul(act_ps[:], ones_col[:], actcol[:],
                                     start=True, stop=True)
                    act_l = route.tile([1, NUM_CHUNKS], dt.float32)
                    nc.scalar.copy(act_l[:], act_ps[:])
                    nc.sync.dma_start(cc_in[:], act_l[:])
                    nc.gpsimd.collective_compute(
                        "AllGather", mybir.AluOpType.bypass,
                        replica_groups=[list(range(N_CORES))],
                        ins=[cc_in.opt()], outs=[cc_out.opt()])

                # prefetches into the collective window
                wf_tiles = []
                for kf in range(KF):
                    w = wfs.tile([P, OUT_F], dt.bfloat16, tag="wf",
                                 name=f"wf{kf}")
                    nc.sync.dma_start(w[:], Wf[kf * P:(kf + 1) * P, :])
                    wf_tiles.append(w)
                bfin_bc = bfinp.tile([P, OUT_F], dt.float32)
                nc.scalar.dma_start(bfin_bc[:], bfb[:])
                bT = route.tile([P, DT_ * NUM_CHUNKS], dt.float32)
                nc.scalar.dma_start(bT[:], bT_d[:])

                # ---- L1 for ALL chunks during the collective window ----
                for c in range(NUM_CHUNKS):
                    hTc = hts.tile([P, DT_ * BS], dt.bfloat16, tag=f"ht{c}",
                                   name=f"ht{c}")
                    for d in range(DT_):
                        ph = ps_h.tile([P, BS], dt.float32, tag="ph",
                                       name=f"ph{c}_{d}")
                        for kt in range(KT):
                            nc.tensor.matmul(
                                ph[:],
                                wcs[c][:, kt * COUT + d * P:
                                       kt * COUT + (d + 1) * P],
                                xcs[c][:, kt * BS:(kt + 1) * BS],
                                start=(kt == 0), stop=(kt == KT - 1))
                        nc.scalar.activation(
                            hTc[:, d * BS:(d + 1) * BS], ph[:],
                            mybir.ActivationFunctionType.Identity,
                            bias=bT[:, d * NUM_CHUNKS + c:
                                    d * NUM_CHUNKS + c + 1])
                    nc.scalar.dma_start(
                        hT_all[c * P:(c + 1) * P, :], hTc[:])

            # ---- phase 2: routing + hT gather + L2 ----
            with tc.tile_pool(name="gath", bufs=1) as gath, \
                 tc.tile_pool(name="outs", bufs=2) as outs, \
                 tc.tile_pool(name="ps_e2", bufs=1, space="PSUM") as ps_e2, \
                 tc.tile_pool(name="ps_o", bufs=6, space="PSUM") as ps_o:

                ag_sb = route.tile([N_CORES, NUM_CHUNKS], dt.float32)
                nc.sync.dma_start(ag_sb[:], cc_out[:])
                # sum the 8 per-core partials AND broadcast to all 128
                # partitions in one matmul: [8,128] ones^T @ [8,8]
                ag_ps = ps_e2.tile([P, NUM_CHUNKS], dt.float32, tag="psg")
                nc.tensor.matmul(ag_ps[:], ones8[:], ag_sb[:],
                                 start=True, stop=True)

                # ------------ top-2, per-partition (identical rows) --------
                maxv = route.tile([P, NUM_CHUNKS], dt.float32)
                maxi = route.tile([P, NUM_CHUNKS], dt.uint32)
                nc.vector.max(maxv[:], ag_ps[:])
                nc.vector.max_index(maxi[:], maxv[:], ag_ps[:])
                maxi_f = route.tile([P, NUM_CHUNKS], dt.float32)
                nc.vector.tensor_copy(maxi_f[:], maxi[:])

                # offsets into hT_all rows: offW[p, s] = sel_s*128 + p
                bc128 = route.tile([P, TOP_K], dt.float32)
                nc.vector.tensor_scalar_mul(bc128[:], maxi_f[:, 0:TOP_K],
                                            float(P))
                offW_f = route.tile([P, TOP_K], dt.float32)
                for s in range(TOP_K):
                    nc.vector.tensor_scalar(
                        offW_f[:, s:s + 1], C_Wf[:],
                        bc128[:, s:s + 1], scalar2=None,
                        op0=mybir.AluOpType.add)
                offW = route.tile([P, TOP_K], dt.int32)
                nc.vector.tensor_copy(offW[:], offW_f[:])

                # ---- gather selected hT rows: one 512KB gather per slot ---
                # hsl[s][p, d*512 + b] = h[feature d*128+p of chunk sel_s, b]
                hsl = []
                for s in range(TOP_K):
                    hs = gath.tile([P, DT_ * BS], dt.bfloat16, tag=f"hs{s}",
                                   name=f"hs{s}")
                    nc.gpsimd.indirect_dma_start(
                        out=hs[:], out_offset=None,
                        in_=hT_all[:],
                        in_offset=bass.IndirectOffsetOnAxis(
                            ap=offW[:, s:s + 1], axis=0))
                    hsl.append(hs)

                def hsel_sl(kf, bt):
                    s, d = divmod(kf, DT_)
                    return hsl[s][:, d * BS + bt * P:d * BS + (bt + 1) * P]

                # ------------ L2: out = h @ W_final + b_final --------------
                # six pre-groups run kf-outer at gather pace, then the rest
                # kf-inner; evict per bt into a [128, 4096] staging row.
                PRE = [(0, 0), (0, 1), (0, 2), (0, 3), (0, 4), (0, 5)]
                pre = {}
                for (bt, o) in PRE:
                    pre[(bt, o)] = ps_o.tile([P, 512], dt.float32, tag="po",
                                             name=f"po_pre{bt}_{o}")
                for kf in range(KF):
                    for (bt, o) in PRE:
                        nc.tensor.matmul(
                            pre[(bt, o)][:],
                            hsel_sl(kf, bt),
                            wf_tiles[kf][:, o * 512:(o + 1) * 512],
                            start=(kf == 0), stop=(kf == KF - 1))

                for bt in range(BT):
                    ot_sb = outs.tile([P, OUT_F], dt.bfloat16, tag="ot",
                                      name=f"ot{bt}")
                    for o in range(OT):
                        osl = slice(o * 512, (o + 1) * 512)
                        if (bt, o) in pre:
                            po = pre[(bt, o)]
                        else:
                            po = ps_o.tile([P, 512], dt.float32, tag="po",
                                           name=f"po{bt}_{o}")
                            for kf in range(KF):
                                nc.tensor.matmul(
                                    po[:], hsel_sl(kf, bt),
                                    wf_tiles[kf][:, osl],
                                    start=(kf == 0), stop=(kf == KF - 1))
                        nc.vector.tensor_tensor(
                            out=ot_sb[:, osl], in0=po[:], in1=bfin_bc[:, osl],
                            op=mybir.AluOpType.add)
                    nc.sync.dma_start(out[bt * P:(bt + 1) * P, :], ot_sb[:])
    nc.compile()
    return nc


def kernel(x, W_chunks, b_chunks, W_final, b_final):
    bf16 = ml_dtypes.bfloat16
    x = np.asarray(x, dtype=np.float32).astype(bf16)
    W_chunks = np.asarray(W_chunks, dtype=np.float32).astype(bf16)
    W_final = np.ascontiguousarray(
        np.asarray(W_final, dtype=np.float32).astype(bf16))
    b_chunks = np.asarray(b_chunks, dtype=np.float32)
    b_final = np.asarray(b_final, dtype=np.float32).reshape(OUT_F)

    # host-side layout prep (input-independent):
    # bT[p, d*8 + c] = b_chunks[c, d*128 + p]
    bT_host = np.ascontiguousarray(
        b_chunks.T.reshape(DT_, P, NUM_CHUNKS).transpose(1, 0, 2)
        .reshape(P, DT_ * NUM_CHUNKS))
    b_final_bc = np.ascontiguousarray(
        np.broadcast_to(b_final[None, :], (P, OUT_F)))
    Wc_rows = np.ascontiguousarray(W_chunks.reshape(IN_F, COUT))

    if "nc" not in _cache:
        _cache["nc"] = _build()
    nc = _cache["nc"]

    in_maps = []
    for c in range(N_CORES):
        xT = np.ascontiguousarray(x[c * BS:(c + 1) * BS].T)  # [4096, 512]
        in_maps.append({
            "xT_shard": xT,
            "Wc_rows": Wc_rows,
            "bT_host": bT_host,
            "W_final": W_final,
            "b_final_bc": b_final_bc,
        })

    res = run_bass_kernel_spmd(nc, in_maps, core_ids=list(range(N_CORES)))
    kernel.last_result = res
    return np.concatenate(
        [res.results[c]["out_shard"].astype(np.float32)
         for c in range(N_CORES)], axis=0)


kernel.last_result = None
